# revision 5
# baseline (speedup 1.0000x reference)
"""Trainium2 Bass kernel for nn_ConditinalBBP (embedding_lookup, memory-bound).

Strategy:
- Data-parallel over batch B=16384 across 8 cores (2048 b-rows each);
  embedding tables replicated per core.
- Key structural wins:
  * w_in / post_in / prior_in depend only on b (the WIN=8 broadcast in the
    reference collapses) -> computed once per b, weighted by WIN on host.
  * All loss terms except the two dot-products enter the mean linearly, so
    only global sums are needed (no per-(b,j) loss materialization).
  * sigma tables: softplus(rho_w) and per-row sum(ln softplus(rho_w)) are
    pure table transforms -> precomputed on host and gathered, removing the
    softplus/ln ACT passes (and all act-table switches between ln and
    softplus sets).
  * ACT table sets: macro-phase A uses {tanh, square, copy}
    (exp_and_others), macro-phase B uses {softplus, square, copy}
    (softplus_and_others) -> exactly one table switch.
- Per core, per group (256 b-rows = 2 sub-tiles of 128):
  Phase A: gather [in_w | sig_in | lsr_in] rows, onehot-matmul for cov_w,
  PE transpose + 3 matmuls for the linear layer, tanh, w_in = tanh + sig*eps.
  Phase B: gather sig_out rows (+lsr), out_w rows (mu, CCE-add fused into
  sig*eps), noise rows; squares/softplus accumulated per partition; dots
  against w_in via free-dim broadcast views; logsigmoid via softplus at end.
- Device ships a [128, 66] per-partition partial-sum tile per core; host
  reduces in float64 and applies wt / mean.
"""
import sys

if "/opt/trn_rl_repo" not in sys.path:
    sys.path.insert(0, "/opt/trn_rl_repo")

import numpy as np

NUM_WORDS = 200000
EMBED = 128
B = 16384
WIN = 8
N_LABELS = 10
NCORES = 8
BC = B // NCORES          # 2048 b-rows per core
NG = 8                    # groups per core; each group = 2 sub-tiles of 128 b
LN_HALF = float(np.log(0.5))

USE_CCE_ADD = True        # fuse w = mu + sig*eps via gather-with-add

_NC_CACHE = {}
_LAST_IN_MAPS = None


def _build_nc():
    import concourse.bacc as bacc
    import concourse.bass as bass
    import concourse.tile as tile
    from concourse import mybir
    from concourse.masks import make_identity

    dt = mybir.dt
    AF = mybir.ActivationFunctionType
    OP = mybir.AluOpType
    AX = mybir.AxisListType

    nc = bacc.Bacc(None, target_bir_lowering=False, debug=False, num_devices=NCORES)

    iwa_d = nc.dram_tensor("iwa", [NUM_WORDS, 260], dt.float32, kind="ExternalInput")
    osig_d = nc.dram_tensor("osig", [NUM_WORDS, 132], dt.float32, kind="ExternalInput")
    ow_d = nc.dram_tensor("ow", [NUM_WORDS, 128], dt.float32, kind="ExternalInput")
    wtt_d = nc.dram_tensor("wtt", [256, 128], dt.float32, kind="ExternalInput")
    covw_d = nc.dram_tensor("covw", [N_LABELS, 128], dt.float32, kind="ExternalInput")
    linb_d = nc.dram_tensor("linb", [1, 128], dt.float32, kind="ExternalInput")
    ones_d = nc.dram_tensor("ones1", [1, 128], dt.float32, kind="ExternalInput")
    idxa_d = nc.dram_tensor("idxa", [128, NG * 2], dt.int32, kind="ExternalInput")
    idxo_d = nc.dram_tensor("idxo", [128, NG * 16], dt.int32, kind="ExternalInput")
    idxn_d = nc.dram_tensor("idxn", [128, NG * 16], dt.int32, kind="ExternalInput")
    oh_d = nc.dram_tensor("ohall", [N_LABELS, NG * 256], dt.float32, kind="ExternalInput")
    epsi_d = nc.dram_tensor("epsi", [128, NG * 256], dt.float32, kind="ExternalInput")
    epso_d = nc.dram_tensor("epso", [128, NG * 2048], dt.float32, kind="ExternalInput")
    acc_d = nc.dram_tensor("acc", [128, 68], dt.float32, kind="ExternalOutput")

    # acc_t column map: 0:8 lnA, 8:16 sqA, 16:24 w2A, 24:32 spA,
    # 32:40 lnB, 40:48 sqB, 48:56 w2B, 56:64 spB, 64 sp1, 65 sp2
    C_LNA, C_SQA, C_W2A, C_SPA = 0, 8, 16, 24
    C_LNB, C_SQB, C_W2B, C_SPB = 32, 40, 48, 56
    C_SP1, C_SP2 = 64, 66

    with tile.TileContext(nc) as tc:
        with (
            tc.tile_pool(name="const", bufs=1) as cp,
            tc.tile_pool(name="resid", bufs=1) as rp,
            tc.tile_pool(name="work", bufs=2) as wp,
            tc.tile_pool(name="psum", bufs=2, space="PSUM") as pp,
        ):
            # ---- constants / whole-kernel loads ----
            wtt0 = cp.tile([128, 128], dt.float32)
            nc.sync.dma_start(out=wtt0[:], in_=wtt_d[0:128, :])
            wtt1 = cp.tile([128, 128], dt.float32)
            nc.sync.dma_start(out=wtt1[:], in_=wtt_d[128:256, :])
            covw = cp.tile([N_LABELS, 128], dt.float32)
            nc.sync.dma_start(out=covw[:], in_=covw_d[:])
            linb = cp.tile([1, 128], dt.float32)
            nc.sync.dma_start(out=linb[:], in_=linb_d[:])
            ones1 = cp.tile([1, 128], dt.float32)
            nc.sync.dma_start(out=ones1[:], in_=ones_d[:])
            ident = cp.tile([128, 128], dt.float32)
            make_identity(nc, ident[:])
            a_idx = cp.tile([128, NG * 2], dt.int32)
            nc.sync.dma_start(out=a_idx[:], in_=idxa_d[:])
            o_idx = cp.tile([128, NG * 16], dt.int32)
            nc.sync.dma_start(out=o_idx[:], in_=idxo_d[:])
            n_idx = cp.tile([128, NG * 16], dt.int32)
            nc.sync.dma_start(out=n_idx[:], in_=idxn_d[:])
            oh_all = cp.tile([N_LABELS, NG * 256], dt.float32)
            nc.sync.dma_start(out=oh_all[:], in_=oh_d[:])

            acc_t = rp.tile([128, 68], dt.float32)
            nc.vector.memset(acc_t[:], 0.0)
            d1acc = rp.tile([128, NG * 16], dt.float32)
            d2acc = rp.tile([128, NG * 16], dt.float32)

            win_ts = []
            w2a_ts = []

            # =================== MACRO PHASE A ===================
            # ACT funcs used: Copy, Tanh, Square (exp_and_others set)
            for g in range(NG):
                iwa_t = wp.tile([128, 520], dt.float32, tag="iwa")
                nc.gpsimd.indirect_dma_start(
                    out=iwa_t[:],
                    out_offset=None,
                    in_=iwa_d[:],
                    in_offset=bass.IndirectOffsetOnAxis(
                        ap=a_idx[:, 2 * g : 2 * g + 2], axis=0
                    ),
                )
                ei_t = wp.tile([128, 256], dt.float32, tag="ei")
                nc.sync.dma_start(
                    out=ei_t[:], in_=epsi_d[:, g * 256 : (g + 1) * 256]
                )
                iwa_r = iwa_t[:].rearrange("p (s q) -> p s q", s=2)
                sig_v = iwa_r[:, :, 128:256]          # [128,2,128]
                lsr_v = iwa_r[:, :, 256:257]          # [128,2,1]
                # sum ln(sig_in) from precomputed per-row sums
                nc.vector.tensor_reduce(
                    out=acc_t[:, C_LNA + g : C_LNA + g + 1],
                    in_=lsr_v,
                    axis=AX.XY,
                    op=OP.add,
                )
                # sum eps_in^2
                scra = wp.tile([128, 256], dt.float32, tag="scra")
                nc.scalar.activation(
                    scra[:], ei_t[:], AF.Square,
                    accum_out=acc_t[:, C_SQA + g : C_SQA + g + 1],
                )
                # t = sig_in * eps_in
                ta = wp.tile([128, 256], dt.float32, tag="ta")
                ta_v = ta[:].rearrange("p (s e) -> p s e", s=2)
                ei_v = ei_t[:].rearrange("p (s e) -> p s e", s=2)
                nc.vector.tensor_tensor(ta_v, sig_v, ei_v, op=OP.mult)

                win_t = rp.tile([128, 256], dt.float32, tag=f"win{g}")
                for s in range(2):
                    mu_v = iwa_r[:, s, 0:128]          # [128,128]
                    pmT = pp.tile([128, 128], dt.float32, tag="pmT")
                    nc.tensor.transpose(out=pmT[:], in_=mu_v, identity=ident[:])
                    muT = wp.tile([128, 128], dt.float32, tag="muT")
                    nc.scalar.copy(muT[:], pmT[:])
                    # host-precomputed onehot of covars -> y^T via matmul
                    pY = pp.tile([128, 128], dt.float32, tag="pY")
                    nc.tensor.matmul(
                        out=pY[:],
                        lhsT=covw[:],
                        rhs=oh_all[:, g * 256 + s * 128 : g * 256 + (s + 1) * 128],
                        start=True,
                        stop=True,
                    )
                    yT = wp.tile([128, 128], dt.float32, tag="yT")
                    nc.scalar.copy(yT[:], pY[:])
                    pW = pp.tile([128, 128], dt.float32, tag="pW")
                    nc.tensor.matmul(
                        out=pW[:], lhsT=muT[:], rhs=wtt0[:], start=True, stop=False
                    )
                    nc.tensor.matmul(
                        out=pW[:], lhsT=yT[:], rhs=wtt1[:], start=False, stop=False
                    )
                    nc.tensor.matmul(
                        out=pW[:], lhsT=ones1[0:1, :], rhs=linb[0:1, :],
                        start=False, stop=True,
                    )
                    nc.scalar.activation(
                        win_t[:, s * 128 : (s + 1) * 128], pW[:], AF.Tanh
                    )
                # w_in = tanh(...) + sig*eps
                nc.vector.tensor_tensor(win_t[:], win_t[:], ta[:], op=OP.add)
                # w_in^2 (kept for deferred softplus) + sum(w_in^2)
                w2a_t = rp.tile([128, 256], dt.float32, tag=f"w2a{g}")
                nc.scalar.activation(
                    w2a_t[:], win_t[:], AF.Square,
                    accum_out=acc_t[:, C_W2A + g : C_W2A + g + 1],
                )
                win_ts.append(win_t)
                w2a_ts.append(w2a_t)

            # =================== MACRO PHASE B ===================
            # ACT funcs used: Softplus, Square, Copy (softplus_and_others)
            for g in range(NG):
                scra2 = wp.tile([128, 256], dt.float32, tag="scra")
                nc.scalar.activation(scra2[:], w2a_ts[g][:], AF.Exp, scale=-12.0)
                scra3 = wp.tile([128, 256], dt.float32, tag="scra3")
                nc.scalar.activation(
                    scra3[:], scra2[:], AF.Ln, bias=1.0,
                    accum_out=acc_t[:, C_SPA + g : C_SPA + g + 1],
                )

            for g in range(NG):
                ost = wp.tile([128, 16 * 132], dt.float32, tag="ost")
                nc.gpsimd.indirect_dma_start(
                    out=ost[:],
                    out_offset=None,
                    in_=osig_d[:],
                    in_offset=bass.IndirectOffsetOnAxis(
                        ap=o_idx[:, 16 * g : 16 * (g + 1)], axis=0
                    ),
                )
                nzt = wp.tile([128, 2048], dt.float32, tag="nzt")
                nc.gpsimd.indirect_dma_start(
                    out=nzt[:],
                    out_offset=None,
                    in_=ow_d[:],
                    in_offset=bass.IndirectOffsetOnAxis(
                        ap=n_idx[:, 16 * g : 16 * (g + 1)], axis=0
                    ),
                )
                eo_t = wp.tile([128, 2048], dt.float32, tag="eo")
                nc.sync.dma_start(
                    out=eo_t[:], in_=epso_d[:, g * 2048 : (g + 1) * 2048]
                )
                ost_r = ost[:].rearrange("p (i q) -> p i q", i=16)
                sig_v = ost_r[:, :, 0:128]             # [128,16,128]
                lsr_v = ost_r[:, :, 128:129]           # [128,16,1]
                nc.vector.tensor_reduce(
                    out=acc_t[:, C_LNB + g : C_LNB + g + 1],
                    in_=lsr_v, axis=AX.XY, op=OP.add,
                )
                scrb = wp.tile([128, 2048], dt.float32, tag="scrb")
                nc.scalar.activation(
                    scrb[:], eo_t[:], AF.Square,
                    accum_out=acc_t[:, C_SQB + g : C_SQB + g + 1],
                )
                # w = sig*eps (+ mu via CCE-add gather)
                wb = wp.tile([128, 2048], dt.float32, tag="wb")
                wb_v = wb[:].rearrange("p (i e) -> p i e", i=16)
                eo_v = eo_t[:].rearrange("p (i e) -> p i e", i=16)
                nc.vector.tensor_tensor(wb_v, sig_v, eo_v, op=OP.mult)
                if USE_CCE_ADD:
                    nc.gpsimd.indirect_dma_start(
                        out=wb[:],
                        out_offset=None,
                        in_=ow_d[:],
                        in_offset=bass.IndirectOffsetOnAxis(
                            ap=o_idx[:, 16 * g : 16 * (g + 1)], axis=0
                        ),
                        compute_op=OP.add,
                    )
                else:
                    mut = wp.tile([128, 2048], dt.float32, tag="mut")
                    nc.gpsimd.indirect_dma_start(
                        out=mut[:],
                        out_offset=None,
                        in_=ow_d[:],
                        in_offset=bass.IndirectOffsetOnAxis(
                            ap=o_idx[:, 16 * g : 16 * (g + 1)], axis=0
                        ),
                    )
                    nc.vector.tensor_tensor(wb[:], wb[:], mut[:], op=OP.add)
                # w^2 + sum(w^2)
                w2b = wp.tile([128, 2048], dt.float32, tag="w2b")
                nc.scalar.activation(
                    w2b[:], wb[:], AF.Square,
                    accum_out=acc_t[:, C_W2B + g : C_W2B + g + 1],
                )
                # sum softplus(-12*w^2) = sum ln(1 + exp(-12*w^2))
                nc.scalar.activation(scrb[:], w2b[:], AF.Exp, scale=-12.0)
                nc.scalar.activation(
                    w2b[:], scrb[:], AF.Ln, bias=1.0,
                    accum_out=acc_t[:, C_SPB + g : C_SPB + g + 1],
                )
                # dots vs w_in (free-dim broadcast of [128, 2, 128] over j=8)
                win_b = (
                    win_ts[g][:]
                    .rearrange("p (s o e) -> p s o e", s=2, o=1)
                    .broadcast_to([128, 2, 8, 128])
                )
                d1 = wp.tile([128, 2048], dt.float32, tag="d1")
                d1_v = d1[:].rearrange("p (s o e) -> p s o e", s=2, o=8)
                wb_v4 = wb[:].rearrange("p (s o e) -> p s o e", s=2, o=8)
                nc.vector.tensor_tensor(d1_v, wb_v4, win_b, op=OP.mult)
                nc.vector.tensor_reduce(
                    out=d1acc[:, 16 * g : 16 * (g + 1)],
                    in_=d1_v, axis=AX.X, op=OP.add,
                )
                d2 = wp.tile([128, 2048], dt.float32, tag="d1")
                d2_v = d2[:].rearrange("p (s o e) -> p s o e", s=2, o=8)
                nz_v4 = nzt[:].rearrange("p (s o e) -> p s o e", s=2, o=8)
                nc.vector.tensor_tensor(d2_v, nz_v4, win_b, op=OP.mult)
                nc.vector.tensor_reduce(
                    out=d2acc[:, 16 * g : 16 * (g + 1)],
                    in_=d2_v, axis=AX.X, op=OP.add,
                )

            # log-sigmoid sums: softplus(x) = relu(x) + ln(1+exp(-|x|))
            scrf = rp.tile([128, NG * 16], dt.float32)
            scrf2 = rp.tile([128, NG * 16], dt.float32)
            for (dacc, sgn, col) in ((d1acc, -1.0, C_SP1), (d2acc, 1.0, C_SP2)):
                nc.scalar.activation(
                    scrf[:], dacc[:], AF.Relu, scale=sgn,
                    accum_out=acc_t[:, col : col + 1],
                )
                nc.scalar.activation(scrf2[:], dacc[:], AF.Abs)
                nc.scalar.activation(scrf[:], scrf2[:], AF.Exp, scale=-1.0)
                nc.scalar.activation(
                    scrf2[:], scrf[:], AF.Ln, bias=1.0,
                    accum_out=acc_t[:, col + 1 : col + 2],
                )
            nc.sync.dma_start(out=acc_d[:], in_=acc_t[:])

    nc.compile()
    return nc


def _get_nc():
    if "nc" not in _NC_CACHE:
        _NC_CACHE["nc"] = _build_nc()
    return _NC_CACHE["nc"]


def _softplus(x):
    return np.logaddexp(0.0, x.astype(np.float64))


def kernel(**inputs):
    from concourse.bass_utils import run_bass_kernel_spmd

    inp = np.asarray(inputs["inputs"]).astype(np.int32).reshape(B)
    outs = np.asarray(inputs["outputs"]).astype(np.int32).reshape(B, WIN)
    cov = np.asarray(inputs["covars"]).astype(np.int32).reshape(B)
    noi = np.asarray(inputs["noise"]).astype(np.int32).reshape(B, WIN)
    wt = float(np.asarray(inputs["wt"]).reshape(-1)[0])
    eps_in = np.asarray(inputs["eps_in"], dtype=np.float32).reshape(B, EMBED)
    eps_out = np.asarray(inputs["eps_out"], dtype=np.float32).reshape(B, WIN * EMBED)
    in_w = np.ascontiguousarray(np.asarray(inputs["in_w"], dtype=np.float32))
    out_w = np.ascontiguousarray(np.asarray(inputs["out_w"], dtype=np.float32))
    in_rho = np.asarray(inputs["in_rho_w"], dtype=np.float32)
    out_rho = np.asarray(inputs["out_rho_w"], dtype=np.float32)
    cov_w = np.ascontiguousarray(np.asarray(inputs["cov_w"], dtype=np.float32))
    lin_w = np.asarray(inputs["lin_w"], dtype=np.float32)
    lin_b = np.asarray(inputs["lin_b"], dtype=np.float32)

    # host-precomputed tables
    sig_in = _softplus(in_rho)
    lsr_in = np.log(sig_in).sum(axis=1)
    iwa = np.zeros((NUM_WORDS, 260), np.float32)
    iwa[:, 0:128] = in_w
    iwa[:, 128:256] = sig_in.astype(np.float32)
    iwa[:, 256] = lsr_in.astype(np.float32)
    sig_out = _softplus(out_rho)
    lsr_out = np.log(sig_out).sum(axis=1)
    osig = np.zeros((NUM_WORDS, 132), np.float32)
    osig[:, 0:128] = sig_out.astype(np.float32)
    osig[:, 128] = lsr_out.astype(np.float32)
    wtt = np.ascontiguousarray(lin_w.T)
    shared = {
        "iwa": iwa,
        "osig": osig,
        "ow": out_w,
        "wtt": wtt,
        "covw": cov_w,
        "linb": np.ascontiguousarray(lin_b.reshape(1, 128)),
        "ones1": np.ones((1, 128), np.float32),
    }

    in_maps = []
    for c in range(NCORES):
        sl = slice(c * BC, (c + 1) * BC)
        m = dict(shared)
        m["idxa"] = np.ascontiguousarray(
            (2 * 0 + inp[sl]).reshape(NG, 2, 128).transpose(2, 0, 1).reshape(128, NG * 2)
        )
        m["idxo"] = np.ascontiguousarray(
            outs[sl].reshape(NG, 2, 128, WIN).transpose(2, 0, 1, 3).reshape(128, NG * 16)
        )
        m["idxn"] = np.ascontiguousarray(
            noi[sl].reshape(NG, 2, 128, WIN).transpose(2, 0, 1, 3).reshape(128, NG * 16)
        )
        covs = cov[sl].reshape(1, NG * 256)
        m["ohall"] = np.ascontiguousarray(
            (covs == np.arange(N_LABELS, dtype=np.int32).reshape(N_LABELS, 1))
            .astype(np.float32)
        )
        m["epsi"] = np.ascontiguousarray(
            eps_in[sl].reshape(NG, 2, 128, 128).transpose(2, 0, 1, 3).reshape(128, -1)
        )
        m["epso"] = np.ascontiguousarray(
            eps_out[sl].reshape(NG, 2, 128, 1024).transpose(2, 0, 1, 3).reshape(128, -1)
        )
        in_maps.append(m)

    global _LAST_IN_MAPS
    _LAST_IN_MAPS = in_maps
    nc = _get_nc()
    res = run_bass_kernel_spmd(nc, in_maps, list(range(NCORES)))

    total = 0.0
    n_a = BC * EMBED            # elements per core in phase A
    n_b = BC * WIN * EMBED      # elements per core in phase B
    for c in range(NCORES):
        a = res.results[c]["acc"].astype(np.float64)
        lnA = a[:, 0:8].sum()
        sqA = a[:, 8:16].sum()
        w2A = a[:, 16:24].sum()
        spA = a[:, 24:32].sum()
        lnB = a[:, 32:40].sum()
        sqB = a[:, 40:48].sum()
        w2B = a[:, 48:56].sum()
        spB = a[:, 56:64].sum()
        sp1 = a[:, 64].sum() + a[:, 65].sum()
        sp2 = a[:, 66].sum() + a[:, 67].sum()
        post_in = -0.5 * sqA - lnA
        prior_in = n_a * LN_HALF - 0.5 * w2A + spA
        post_out = -0.5 * sqB - lnB
        prior_out = n_b * LN_HALF - 0.5 * w2B + spB
        lik = -(sp1 + sp2)
        total += wt * (
            WIN * (post_in - prior_in) + post_out - prior_out
        ) - lik
    return np.float32(total / (B * WIN))
